# revision 7
# baseline (speedup 1.0000x reference)
import numpy as np

import concourse.bass as bass
import concourse.mybir as mybir
from concourse.tile import TileContext
from concourse.bass_utils import run_bass_kernel_spmd

B, L, D = 2, 2048, 1024
H, KV, DH = 16, 4, 64
G = H // KV
LT = L // 128
DT = D // 128
QW = 512
NQ = L // QW
F32 = mybir.dt.float32
F32R = mybir.dt.float32r

_wsplit_ctr = [0]


def _legalize_sync_waits(nc, max_waits=1):
    for fn in nc.m.functions:
        for blk in fn.blocks:
            out = []
            changed = False
            for inst in blk.instructions:
                si = inst.sync_info
                waits = list(si.on_wait) if si is not None else []
                if len(waits) > max_waits:
                    extra, keep = waits[:-max_waits], waits[-max_waits:]
                    for i in range(0, len(extra), max_waits):
                        _wsplit_ctr[0] += 1
                        nop = mybir.InstNoOp(
                            name=f"I-wsplit-{_wsplit_ctr[0]}", ins=[], outs=[]
                        )
                        nop.engine = inst.engine
                        nop.sync_info = mybir.SyncInfo(
                            on_wait=extra[i : i + max_waits], on_update=[]
                        )
                        out.append(nop)
                    inst.sync_info = mybir.SyncInfo(
                        on_wait=keep, on_update=list(si.on_update)
                    )
                    changed = True
                out.append(inst)
            if changed:
                blk.instructions = out


def build_nc():
    nc = bass.Bass()
    xb = nc.dram_tensor("xb", [L, D], F32, kind="ExternalInput")
    wq = nc.dram_tensor("wq", [D, G * DH], F32, kind="ExternalInput")
    wk = nc.dram_tensor("wk", [D, DH], F32, kind="ExternalInput")
    wv = nc.dram_tensor("wv", [D, DH], F32, kind="ExternalInput")
    wo = nc.dram_tensor("wo", [G * DH, D], F32, kind="ExternalInput")
    cos = nc.dram_tensor("cos", [L, DH // 2], F32, kind="ExternalInput")
    sin = nc.dram_tensor("sin", [L, DH // 2], F32, kind="ExternalInput")
    part = nc.dram_tensor("part", [L, D], F32, kind="ExternalOutput")

    with TileContext(nc) as tc:
        with tc.tile_pool(name="persist", bufs=1) as persist:
            ident = persist.tile([128, 128], F32)
            nc.gpsimd.memset(ident, 0.0)
            nc.gpsimd.affine_select(
                out=ident, in_=ident, compare_op=mybir.AluOpType.not_equal,
                fill=1.0, base=0, pattern=[[-1, 128]], channel_multiplier=1,
            )
            identr = persist.tile([128, 128], F32R)
            nc.vector.tensor_copy(out=identr, in_=ident)
            onesf = persist.tile([128, 1], F32)
            nc.vector.memset(onesf, 1.0)
            ones64r = persist.tile([1, 64], F32R)
            nc.vector.tensor_copy(
                out=ones64r, in_=onesf[0:1, 0:1].broadcast_to([1, 64])
            )

            cmask = [persist.tile([128, QW], F32, name=f"cmask{j}", tag=f"cmask{j}") for j in range(4)]
            for j in range(4):
                nc.gpsimd.memset(cmask[j], 1.0)
                nc.gpsimd.affine_select(
                    out=cmask[j], in_=cmask[j],
                    compare_op=mybir.AluOpType.is_ge,
                    fill=0.0, base=-j * 128,
                    pattern=[[1, QW]], channel_multiplier=-1,
                )

            wqkv = persist.tile([128, DT, 384], F32R)
            wo_sb = persist.tile([128, 2, D], F32R)
            cos_sb = persist.tile([128, LT, DH // 2], F32)
            sin_sb = persist.tile([128, LT, DH // 2], F32)
            nc.sync.dma_start(
                out=cos_sb, in_=cos[:, :].rearrange("(t p) d -> p t d", p=128)
            )
            nc.sync.dma_start(
                out=sin_sb, in_=sin[:, :].rearrange("(t p) d -> p t d", p=128)
            )
            with tc.tile_pool(name="wstage", bufs=1) as wstage:
                wqkv_f = wstage.tile([128, DT, 384], F32)
                nc.sync.dma_start(
                    out=wqkv_f[:, :, 0 : G * DH],
                    in_=wq[:, :].rearrange("(t p) m -> p t m", p=128),
                )
                nc.sync.dma_start(
                    out=wqkv_f[:, :, G * DH : G * DH + DH],
                    in_=wk[:, :].rearrange("(t p) m -> p t m", p=128),
                )
                nc.sync.dma_start(
                    out=wqkv_f[:, :, G * DH + DH : 384],
                    in_=wv[:, :].rearrange("(t p) m -> p t m", p=128),
                )
                nc.vector.tensor_copy(out=wqkv, in_=wqkv_f)
                wo_f = wstage.tile([128, 2, D], F32)
                nc.sync.dma_start(
                    out=wo_f, in_=wo[:, :].rearrange("(t p) m -> p t m", p=128)
                )
                nc.vector.tensor_copy(out=wo_sb, in_=wo_f)

            qkv_sb = persist.tile([128, LT, 384], F32R)

            with (
                tc.tile_pool(name="xstream", bufs=5) as xstream,
                tc.tile_pool(name="xT", bufs=1) as xT_pool,
                tc.tile_pool(name="ps_a", bufs=3, space="PSUM") as ps_a,
                tc.tile_pool(name="ps_c", bufs=2, space="PSUM") as ps_c,
            ):
                xT = [xT_pool.tile([128, L], F32R, name=f"xT{d}", tag=f"xT{d}") for d in range(DT)]

                for ltg in range(LT // 4):
                    xtiles = []
                    for i in range(4):
                        lt = ltg * 4 + i
                        xt = xstream.tile([128, D], F32, tag="xload")
                        nc.sync.dma_start(
                            out=xt, in_=xb[lt * 128 : (lt + 1) * 128, :]
                        )
                        xtiles.append(xt)
                    for d in range(DT):
                        pst = ps_a.tile([128, 512], F32)
                        for i in range(4):
                            nc.tensor.transpose(
                                out=pst[:, i * 128 : (i + 1) * 128],
                                in_=xtiles[i][:, d * 128 : (d + 1) * 128],
                                identity=ident,
                            )
                        nc.vector.tensor_copy(
                            out=xT[d][:, ltg * 512 : (ltg + 1) * 512], in_=pst
                        )

                for lt in range(LT):
                    ps = ps_c.tile([128, 384], F32)
                    for d in range(DT):
                        nc.tensor.matmul(
                            ps,
                            lhsT=xT[d][:, lt * 128 : (lt + 1) * 128],
                            rhs=wqkv[:, d, :],
                            start=(d == 0),
                            stop=(d == DT - 1),
                        )
                    nc.vector.tensor_copy(out=qkv_sb[:, lt, :], in_=ps)

            with tc.tile_pool(name="rope", bufs=2) as rope_pool:
                for ltg in range(LT // 4):
                    lts = slice(ltg * 4, ltg * 4 + 4)
                    grp = qkv_sb[:, lts, :].rearrange("p t (u x) -> p t u x", x=32)
                    q1 = grp[:, :, 0:10:2, :]
                    q2 = grp[:, :, 1:11:2, :]
                    c = cos_sb[:, lts, :].unsqueeze(2).broadcast_to([128, 4, 5, 32])
                    s = sin_sb[:, lts, :].unsqueeze(2).broadcast_to([128, 4, 5, 32])
                    t1 = rope_pool.tile([128, 4, 5, 32], F32, tag="t1")
                    t2 = rope_pool.tile([128, 4, 5, 32], F32, tag="t2")
                    nc.vector.tensor_mul(t1, q1, s)
                    nc.vector.tensor_mul(q1, q1, c)
                    nc.vector.tensor_mul(t2, q2, s)
                    nc.vector.tensor_sub(q1, q1, t2)
                    nc.vector.tensor_mul(q2, q2, c)
                    nc.vector.tensor_add(q2, q2, t1)

            qt = [persist.tile([128, L], F32R, name=f"qt{j}", tag=f"qt{j}") for j in range(2)]
            kt2 = persist.tile([128, L], F32R)
            v_sb = persist.tile([128, LT, DH + 1], F32R)
            nc.vector.tensor_copy(
                out=v_sb[:, :, DH : DH + 1],
                in_=onesf.unsqueeze(1).broadcast_to([128, LT, 1]),
            )
            for lt in range(LT):
                nc.vector.tensor_copy(
                    out=v_sb[:, lt, 0:DH], in_=qkv_sb[:, lt, 320:384]
                )
            with tc.tile_pool(name="ps_e", bufs=3, space="PSUM") as ps_e:
                for u in range(5):
                    for ltg in range(LT // 4):
                        pse = ps_e.tile([64, 512], F32R)
                        for i in range(4):
                            lt = ltg * 4 + i
                            nc.tensor.transpose(
                                out=pse[:, i * 128 : (i + 1) * 128],
                                in_=qkv_sb[:, lt, u * 64 : (u + 1) * 64],
                                identity=identr,
                            )
                        if u < G:
                            nc.vector.tensor_copy(
                                out=qt[u // 2][
                                    (u % 2) * 64 : (u % 2) * 64 + 64,
                                    ltg * 512 : (ltg + 1) * 512,
                                ],
                                in_=pse,
                            )
                        else:
                            nc.vector.tensor_copy(
                                out=kt2[0:64, ltg * 512 : (ltg + 1) * 512], in_=pse
                            )
                nc.vector.tensor_copy(out=kt2[64:128, :], in_=kt2[0:64, :])

            ot = [persist.tile([128, L], F32R, name=f"ot{m}", tag=f"ot{m}") for m in range(2)]
            with (
                tc.tile_pool(name="ps_s", bufs=3, space="PSUM") as ps_s,
                tc.tile_pool(name="ps_o", bufs=2, space="PSUM") as ps_o,
                tc.tile_pool(name="ps_bc", bufs=1, space="PSUM") as ps_bc,
                tc.tile_pool(name="pt", bufs=4) as pt_pool,
                tc.tile_pool(name="small", bufs=4) as small,
                tc.tile_pool(name="ps_g", bufs=2, space="PSUM") as ps_g,
                tc.tile_pool(name="ostage", bufs=3) as ostage,
            ):
                for h in range(G):
                    hb = (h % 2) * 64
                    for qi in range(NQ):
                        nki = 4 * (qi + 1)
                        ops = ps_o.tile([DH + 1, QW], F32)
                        for ki in range(nki):
                            pss = ps_s.tile([128, QW], F32)
                            nc.tensor.matmul(
                                pss,
                                lhsT=kt2[hb : hb + 64, ki * 128 : (ki + 1) * 128],
                                rhs=qt[h // 2][hb : hb + 64, qi * QW : (qi + 1) * QW],
                                start=True,
                                stop=True,
                            )
                            pt = pt_pool.tile([128, QW], F32R, tag="pt")
                            nc.scalar.activation(
                                out=pt, in_=pss,
                                func=mybir.ActivationFunctionType.Exp,
                                scale=0.125,
                            )
                            j = ki - 4 * qi
                            if j >= 0:
                                nc.vector.tensor_mul(pt, pt, cmask[j])
                            nc.tensor.matmul(
                                ops,
                                lhsT=v_sb[:, ki, :],
                                rhs=pt,
                                start=(ki == 0),
                                stop=(ki == nki - 1),
                            )
                        recip = small.tile([1, QW], F32R, tag="recip")
                        with nc.allow_low_precision(reason="f32r softmax denom"):
                            nc.vector.reciprocal(recip, ops[DH : DH + 1, :])
                        bc = ps_bc.tile([64, QW], F32)
                        nc.tensor.matmul(
                            bc, lhsT=ones64r, rhs=recip, start=True, stop=True
                        )
                        bc_sb = small.tile([64, QW], F32, tag="bc_sb")
                        nc.vector.tensor_copy(out=bc_sb, in_=bc)
                        nc.vector.tensor_mul(
                            ot[h // 2][hb : hb + 64, qi * QW : (qi + 1) * QW],
                            ops[0:DH, :],
                            bc_sb,
                        )

                for lt in range(LT):
                    for half in range(2):
                        psg = ps_g.tile([128, 512], F32)
                        for m in range(2):
                            nc.tensor.matmul(
                                psg,
                                lhsT=ot[m][:, lt * 128 : (lt + 1) * 128],
                                rhs=wo_sb[:, m, half * 512 : (half + 1) * 512],
                                start=(m == 0),
                                stop=(m == 1),
                            )
                        og = ostage.tile([128, 512], F32, tag="og")
                        nc.vector.tensor_copy(out=og, in_=psg)
                        nc.sync.dma_start(
                            out=part[lt * 128 : (lt + 1) * 128,
                                     half * 512 : (half + 1) * 512],
                            in_=og,
                        )

    _legalize_sync_waits(nc)
    return nc


def _in_maps(x, rope_cos, rope_sin, Wq, Wk, Wv, Wo):
    maps = []
    for c in range(8):
        b, g = c // 4, c % 4
        maps.append({
            "xb": np.ascontiguousarray(x[b]),
            "wq": np.ascontiguousarray(Wq[:, g * 256 : (g + 1) * 256]),
            "wk": np.ascontiguousarray(Wk[:, g * 64 : (g + 1) * 64]),
            "wv": np.ascontiguousarray(Wv[:, g * 64 : (g + 1) * 64]),
            "wo": np.ascontiguousarray(Wo[g * 256 : (g + 1) * 256, :]),
            "cos": np.ascontiguousarray(rope_cos),
            "sin": np.ascontiguousarray(rope_sin),
        })
    return maps


def _gather(results):
    out = np.zeros((B, L, D), dtype=np.float32)
    for c in range(8):
        out[c // 4] += results[c]["part"]
    return out


def kernel(x, rope_cos, rope_sin, Wq, Wk, Wv, Wo, **run_kwargs):
    args = [
        np.asarray(a, dtype=np.float32)
        for a in (x, rope_cos, rope_sin, Wq, Wk, Wv, Wo)
    ]
    nc = build_nc()
    res = run_bass_kernel_spmd(
        nc, _in_maps(*args), core_ids=list(range(8)), **run_kwargs
    )
    out = _gather(res.results)
    if run_kwargs:
        return out, res
    return out
